# revision 22
# baseline (speedup 1.0000x reference)
"""Trainium2 Bass kernel for nn_LocalConnectivity (diamond-ring circular stencil).

out[i,j] = sum_{d=1..5} w_d * sum_{|di|+|dj|=d} x[(i+di)%H, (j+dj)%W]

Strategy: row-shard across 8 NeuronCores (512 rows each + 5-row circular
halo, columns pre-padded with 5-col circular halo on host). Per core the
60-tap stencil runs on the TensorEngine as banded matmuls. The kernel is
symmetric in dj, so the DVE/Pool engines pre-fold the column shifts
(S_j = x(c-j) + x(c+j)) and only 6 matmul streams per 512-col chunk are
needed (dj=0 plus folded j=1..5) instead of 11. All matmul operands are
bf16 (1 cycle/row at the full 2.4 GHz PE clock vs fp32r's 1.2 GHz).
Output is written bf16 and upcast on host. Stores are issued per 512-col
chunk (strided DRAM destination) so the DGE round-robins packets across
all 16 SDMA engines instead of chaining the whole window on one.
"""
import numpy as np
import ml_dtypes
from contextlib import ExitStack

import concourse.bass as bass
import concourse.tile as tile
from concourse import bacc, mybir
from concourse.bass_utils import run_bass_kernel_spmd

N_CORES = 8
H = W = 4096
MAXD = 5
ROWS_PER_CORE = H // N_CORES          # 512
IN_ROWS = ROWS_PER_CORE + 2 * MAXD    # 522
IN_COLS = W + 2 * MAXD                # 4106
NCOL = 512                            # matmul free dim (one PSUM bank, fp32 max)
NCHUNK = W // NCOL                    # 8
M_OUT = 118                           # output rows per row-window (K=128 - 2*MAXD)
NJ = MAXD + 1                         # dj=0 plus folded |dj|=1..5
# row windows: (input_row_start, out_row_start, K, M)
WINDOWS = []
_o = 0
while _o < ROWS_PER_CORE:
    m = min(M_OUT, ROWS_PER_CORE - _o)
    WINDOWS.append((_o, _o, m + 2 * MAXD, m))
    _o += m

_CACHE = {}


def _band_weights(distance_weights: np.ndarray) -> np.ndarray:
    """w_flat [128, 6*118] bf16: w_flat[k, j*118 + m] = K2d[k-m-5, j].

    Column block j holds the vertical taps for |dj|=j (the dj fold uses
    K2d[di, dj] == K2d[di, -dj], so one band serves both signs)."""
    wd = np.asarray(distance_weights, dtype=np.float32)
    w = np.zeros((NJ, 128, M_OUT), dtype=np.float32)
    for dj in range(0, MAXD + 1):
        for di in range(-MAXD, MAXD + 1):
            d = abs(di) + dj
            if not (1 <= d <= MAXD):
                continue
            m = np.arange(M_OUT)
            k = m + MAXD + di
            ok = (k >= 0) & (k < 128)
            w[dj, k[ok], m[ok]] = wd[d - 1]
    out = np.ascontiguousarray(w.transpose(1, 0, 2).reshape(128, NJ * M_OUT))
    return out.astype(ml_dtypes.bfloat16)


def _build():
    dtb = mybir.dt.bfloat16
    dtf = mybir.dt.float32
    nc = bacc.Bacc("TRN2", target_bir_lowering=False, debug=False,
                   num_devices=N_CORES)
    x = nc.dram_tensor("x", [IN_ROWS, IN_COLS], dtb, kind="ExternalInput").ap()
    wts = nc.dram_tensor("w", [128, NJ * M_OUT], dtb, kind="ExternalInput").ap()
    y = nc.dram_tensor("y", [ROWS_PER_CORE, W], dtb, kind="ExternalOutput").ap()

    with tile.TileContext(nc) as tc, ExitStack() as ctx:
        spool = ctx.enter_context(tc.tile_pool(name="strip", bufs=3))
        fpool = ctx.enter_context(tc.tile_pool(name="folds", bufs=2))
        wpool = ctx.enter_context(tc.tile_pool(name="wts", bufs=1))
        ocpool = ctx.enter_context(tc.tile_pool(name="outc", bufs=16))
        ppool = ctx.enter_context(tc.tile_pool(name="ps", bufs=8, space="PSUM"))

        # Strip loads ride the scalar HWDGE queue: one contiguous full-row
        # transfer per window; row packets round-robin over 16 SDMA engines.
        strips = {}
        all_folds = {}

        def load_strip(wi, engine=None):
            in0, _, kdim, _ = WINDOWS[wi]
            st = spool.tile([128, IN_COLS], dtb, tag="strip")
            (engine or nc.scalar).dma_start(st[:kdim, :], x[in0:in0 + kdim, :])
            strips[wi] = st

        def fold(wi, maxj=MAXD):
            # Column folds S_j = x(c-j) + x(c+j), all on DVE (a concurrent
            # Pool-engine tensor op would contend on the SBUF bus and slow
            # both engines ~4x). Issued one window ahead so the PE rarely
            # waits. Windows 0/1 skip the S_5 fold (the PE runs +-5 streams
            # instead): DVE is saturated from strip0-arrival through window
            # 1, and shedding 2.28us of fold there beats the 1.73us extra
            # stream on the PE.
            st = strips[wi]
            in0, _, kdim, _ = WINDOWS[wi]
            fs = [st]
            for j in range(1, maxj + 1):
                sj = fpool.tile([128, W], dtb, tag=f"s{j}")
                nc.vector.tensor_add(sj[:kdim, :],
                                     st[:kdim, MAXD - j:MAXD - j + W],
                                     st[:kdim, MAXD + j:MAXD + j + W])
                fs.append(sj)
            all_folds[wi] = fs

        # Startup critical path: the first matmul needs wt + all of strip0.
        # The sync queue drains on only 2 SDMA engines (wt there lands at
        # ~12us), so wt rides gpsimd ahead of strip1 instead.
        load_strip(0)
        wt = wpool.tile([128, NJ * M_OUT], dtb)
        nc.gpsimd.dma_start(wt[:], wts[:])
        load_strip(1, engine=nc.gpsimd)
        fold(0, maxj=MAXD - 1)

        NW = len(WINDOWS)
        for wi, (in0, out0, kdim, m) in enumerate(WINDOWS):
            st = strips.pop(wi)
            if wi + 1 < NW:
                fold(wi + 1, maxj=MAXD - 1 if wi == 0 else MAXD)
            folds = all_folds.pop(wi)
            last = wi == NW - 1
            # (j, source-slice) streams per chunk. Windows 0/1: j=5 runs as
            # a +-5 stream pair off the raw strip (the folded band serves
            # both signs since K2d is dj-symmetric).
            def streams(c0):
                ss = [(0, st[:kdim, MAXD + c0:MAXD + c0 + NCOL])]
                for j in range(1, len(folds)):
                    ss.append((j, folds[j][:kdim, c0:c0 + NCOL]))
                if len(folds) == MAXD:
                    ss.append((MAXD, st[:kdim, c0:c0 + NCOL]))
                    ss.append((MAXD, st[:kdim, 2 * MAXD + c0:
                                        2 * MAXD + c0 + NCOL]))
                return ss
            pss = [ppool.tile([m, NCOL], dtf, tag="ps", name=f"ps{cc}")
                   for cc in range(NCHUNK)]
            if wi <= 1:
                # dj-outer: each j-pass (8 chunks, ~1.8us) consumes S_j
                # right as DVE finishes folding it -- the PE trails the fold
                # pipeline by one j instead of waiting for the whole fold
                # set. All 8 PSUM banks accumulate at once.
                chunk_streams = [streams(cc * NCOL) for cc in range(NCHUNK)]
                ns = len(chunk_streams[0])
                for si in range(ns):
                    for cc in range(NCHUNK):
                        j, src = chunk_streams[cc][si]
                        nc.tensor.matmul(
                            pss[cc][:], wt[:kdim, j * M_OUT:j * M_OUT + m],
                            src, start=(si == 0), stop=(si == ns - 1),
                            skip_group_check=True,
                        )
            else:
                for cc in range(NCHUNK):
                    ss = streams(cc * NCOL)
                    for si, (j, src) in enumerate(ss):
                        nc.tensor.matmul(
                            pss[cc][:], wt[:kdim, j * M_OUT:j * M_OUT + m],
                            src, start=(si == 0), stop=(si == len(ss) - 1),
                        )
            for cc2 in range(0, NCHUNK, 2):
                # Copy two PSUM banks into one [m, 1024] tile and store both
                # at once: 2KB-row store packets double SDMA efficiency and
                # halve the SWDGE descriptor-generation load on Pool. In the
                # last window DVE is done folding, so it takes half the
                # copies to cut the drain tail.
                ob = ocpool.tile([m, 2 * NCOL], dtb, tag="out")
                for h in range(2):
                    if last and h == 1:
                        nc.vector.tensor_copy(
                            ob[:, h * NCOL:(h + 1) * NCOL], pss[cc2 + h][:])
                    else:
                        nc.scalar.copy(
                            ob[:, h * NCOL:(h + 1) * NCOL], pss[cc2 + h][:])
                nc.gpsimd.dma_start(
                    y[out0:out0 + m, cc2 * NCOL:(cc2 + 2) * NCOL], ob[:])
            # Prefetch two windows ahead, emitted AFTER this window's copies:
            # the Act sequencer is in-order, so the strip's descriptor-gen
            # queues behind them and its packets stop competing with the
            # startup-critical strip0 on the shared SDMA ring.
            if wi + 2 < NW:
                load_strip(wi + 2)
    nc.compile()
    return nc


def make_in_maps(grid_spikes: np.ndarray, distance_weights: np.ndarray):
    x = np.asarray(grid_spikes, dtype=np.float32).astype(ml_dtypes.bfloat16)
    assert x.shape == (H, W)
    w_flat = _band_weights(np.asarray(distance_weights, dtype=np.float32))
    xpad = np.concatenate([x[:, -MAXD:], x, x[:, :MAXD]], axis=1)
    in_maps = []
    for c in range(N_CORES):
        rows = np.arange(c * ROWS_PER_CORE - MAXD,
                         c * ROWS_PER_CORE + ROWS_PER_CORE + MAXD) % H
        in_maps.append({"x": np.ascontiguousarray(xpad[rows]), "w": w_flat})
    return in_maps


def kernel(grid_spikes: np.ndarray, distance_weights: np.ndarray) -> np.ndarray:
    if "nc" not in _CACHE:
        _CACHE["nc"] = _build()
    nc = _CACHE["nc"]
    in_maps = make_in_maps(grid_spikes, distance_weights)
    res = run_bass_kernel_spmd(nc, in_maps, list(range(N_CORES)))
    out = np.concatenate([res.results[c]["y"] for c in range(N_CORES)], axis=0)
    return out.astype(np.float32)


# revision 23
# speedup vs baseline: 1.1742x; 1.1742x over previous
"""Trainium2 Bass kernel for nn_LocalConnectivity (diamond-ring circular stencil).

out[i,j] = sum_{d=1..5} w_d * sum_{|di|+|dj|=d} x[(i+di)%H, (j+dj)%W]

Strategy: row-shard across 8 NeuronCores (512 rows each + 5-row circular
halo, columns pre-padded with 5-col circular halo on host). Per core the
60-tap stencil runs on the TensorEngine as banded matmuls. The kernel is
symmetric in dj, so the DVE/Pool engines pre-fold the column shifts
(S_j = x(c-j) + x(c+j)) and only 6 matmul streams per 512-col chunk are
needed (dj=0 plus folded j=1..5) instead of 11. All matmul operands are
bf16 (1 cycle/row at the full 2.4 GHz PE clock vs fp32r's 1.2 GHz).
Output is written bf16 and upcast on host. Stores are issued per 512-col
chunk (strided DRAM destination) so the DGE round-robins packets across
all 16 SDMA engines instead of chaining the whole window on one.
"""
import numpy as np
import ml_dtypes
from contextlib import ExitStack

import concourse.bass as bass
import concourse.tile as tile
from concourse import bacc, mybir
from concourse.bass_utils import run_bass_kernel_spmd

N_CORES = 8
H = W = 4096
MAXD = 5
ROWS_PER_CORE = H // N_CORES          # 512
IN_ROWS = ROWS_PER_CORE + 2 * MAXD    # 522
IN_COLS = W + 2 * MAXD                # 4106
NCOL = 512                            # matmul free dim (one PSUM bank, fp32 max)
NCHUNK = W // NCOL                    # 8
M_OUT = 118                           # output rows per row-window (K=128 - 2*MAXD)
NJ = MAXD + 1                         # dj=0 plus folded |dj|=1..5
# row windows: (input_row_start, out_row_start, K, M)
WINDOWS = []
_o = 0
while _o < ROWS_PER_CORE:
    m = min(M_OUT, ROWS_PER_CORE - _o)
    WINDOWS.append((_o, _o, m + 2 * MAXD, m))
    _o += m

_CACHE = {}


def _band_weights(distance_weights: np.ndarray) -> np.ndarray:
    """w_flat [128, 6*118] bf16: w_flat[k, j*118 + m] = K2d[k-m-5, j].

    Column block j holds the vertical taps for |dj|=j (the dj fold uses
    K2d[di, dj] == K2d[di, -dj], so one band serves both signs)."""
    wd = np.asarray(distance_weights, dtype=np.float32)
    w = np.zeros((NJ, 128, M_OUT), dtype=np.float32)
    for dj in range(0, MAXD + 1):
        for di in range(-MAXD, MAXD + 1):
            d = abs(di) + dj
            if not (1 <= d <= MAXD):
                continue
            m = np.arange(M_OUT)
            k = m + MAXD + di
            ok = (k >= 0) & (k < 128)
            w[dj, k[ok], m[ok]] = wd[d - 1]
    out = np.ascontiguousarray(w.transpose(1, 0, 2).reshape(128, NJ * M_OUT))
    return out.astype(ml_dtypes.bfloat16)


def _build():
    dtb = mybir.dt.bfloat16
    dtf = mybir.dt.float32
    nc = bacc.Bacc("TRN2", target_bir_lowering=False, debug=False,
                   num_devices=N_CORES)
    x = nc.dram_tensor("x", [IN_ROWS, IN_COLS], dtb, kind="ExternalInput").ap()
    wts = nc.dram_tensor("w", [128, NJ * M_OUT], dtb, kind="ExternalInput").ap()
    y = nc.dram_tensor("y", [ROWS_PER_CORE, W], dtb, kind="ExternalOutput").ap()

    with tile.TileContext(nc) as tc, ExitStack() as ctx:
        spool = ctx.enter_context(tc.tile_pool(name="strip", bufs=3))
        fpool = ctx.enter_context(tc.tile_pool(name="folds", bufs=2))
        wpool = ctx.enter_context(tc.tile_pool(name="wts", bufs=1))
        ocpool = ctx.enter_context(tc.tile_pool(name="outc", bufs=16))
        ppool = ctx.enter_context(tc.tile_pool(name="ps", bufs=8, space="PSUM"))

        # Strip loads ride the scalar HWDGE queue: one contiguous full-row
        # transfer per window; row packets round-robin over 16 SDMA engines.
        strips = {}
        all_folds = {}

        def load_strip(wi, engine=None):
            in0, _, kdim, _ = WINDOWS[wi]
            st = spool.tile([128, IN_COLS], dtb, tag="strip")
            (engine or nc.scalar).dma_start(st[:kdim, :], x[in0:in0 + kdim, :])
            strips[wi] = st

        def fold(wi, maxj=MAXD):
            # Column folds S_j = x(c-j) + x(c+j), all on DVE (a concurrent
            # Pool-engine tensor op would contend on the SBUF bus and slow
            # both engines ~4x). Issued one window ahead so the PE rarely
            # waits. Windows 0/1 skip the S_5 fold (the PE runs +-5 streams
            # instead): DVE is saturated from strip0-arrival through window
            # 1, and shedding 2.28us of fold there beats the 1.73us extra
            # stream on the PE.
            st = strips[wi]
            in0, _, kdim, _ = WINDOWS[wi]
            fs = [st]
            for j in range(1, maxj + 1):
                sj = fpool.tile([128, W], dtb, tag=f"s{j}")
                nc.vector.tensor_add(sj[:kdim, :],
                                     st[:kdim, MAXD - j:MAXD - j + W],
                                     st[:kdim, MAXD + j:MAXD + j + W])
                fs.append(sj)
            all_folds[wi] = fs

        # Startup critical path: the first matmul needs wt + all of strip0.
        # The sync queue drains on only 2 SDMA engines (wt there lands at
        # ~12us), so wt rides gpsimd ahead of strip1 instead.
        load_strip(0)
        wt = wpool.tile([128, NJ * M_OUT], dtb)
        nc.gpsimd.dma_start(wt[:], wts[:])
        load_strip(1, engine=nc.gpsimd)
        fold(0, maxj=MAXD - 1)

        NW = len(WINDOWS)
        for wi, (in0, out0, kdim, m) in enumerate(WINDOWS):
            st = strips.pop(wi)
            if wi + 2 < NW:
                load_strip(wi + 2)
            if wi + 1 < NW:
                fold(wi + 1, maxj=MAXD - 1 if wi == 0 else MAXD)
            folds = all_folds.pop(wi)
            last = wi == NW - 1
            # (j, source-slice) streams per chunk. Windows 0/1: j=5 runs as
            # a +-5 stream pair off the raw strip (the folded band serves
            # both signs since K2d is dj-symmetric).
            def streams(c0):
                ss = [(0, st[:kdim, MAXD + c0:MAXD + c0 + NCOL])]
                for j in range(1, len(folds)):
                    ss.append((j, folds[j][:kdim, c0:c0 + NCOL]))
                if len(folds) == MAXD:
                    ss.append((MAXD, st[:kdim, c0:c0 + NCOL]))
                    ss.append((MAXD, st[:kdim, 2 * MAXD + c0:
                                        2 * MAXD + c0 + NCOL]))
                return ss
            pss = [ppool.tile([m, NCOL], dtf, tag="ps", name=f"ps{cc}")
                   for cc in range(NCHUNK)]
            if wi <= 1:
                # dj-outer: each j-pass (8 chunks, ~1.8us) consumes S_j
                # right as DVE finishes folding it -- the PE trails the fold
                # pipeline by one j instead of waiting for the whole fold
                # set. All 8 PSUM banks accumulate at once.
                chunk_streams = [streams(cc * NCOL) for cc in range(NCHUNK)]
                ns = len(chunk_streams[0])
                for si in range(ns):
                    for cc in range(NCHUNK):
                        j, src = chunk_streams[cc][si]
                        nc.tensor.matmul(
                            pss[cc][:], wt[:kdim, j * M_OUT:j * M_OUT + m],
                            src, start=(si == 0), stop=(si == ns - 1),
                            skip_group_check=True,
                        )
            else:
                for cc in range(NCHUNK):
                    ss = streams(cc * NCOL)
                    for si, (j, src) in enumerate(ss):
                        nc.tensor.matmul(
                            pss[cc][:], wt[:kdim, j * M_OUT:j * M_OUT + m],
                            src, start=(si == 0), stop=(si == len(ss) - 1),
                        )
            for cc2 in range(0, NCHUNK, 2):
                # Copy two PSUM banks into one [m, 1024] tile and store both
                # at once: 2KB-row store packets double SDMA efficiency and
                # halve the SWDGE descriptor-generation load on Pool. In the
                # last window DVE is done folding, so it takes half the
                # copies to cut the drain tail.
                ob = ocpool.tile([m, 2 * NCOL], dtb, tag="out")
                for h in range(2):
                    if last and h == 1:
                        nc.vector.tensor_copy(
                            ob[:, h * NCOL:(h + 1) * NCOL], pss[cc2 + h][:])
                    else:
                        nc.scalar.copy(
                            ob[:, h * NCOL:(h + 1) * NCOL], pss[cc2 + h][:])
                nc.gpsimd.dma_start(
                    y[out0:out0 + m, cc2 * NCOL:(cc2 + 2) * NCOL], ob[:])
    nc.compile()
    return nc


def make_in_maps(grid_spikes: np.ndarray, distance_weights: np.ndarray):
    x = np.asarray(grid_spikes, dtype=np.float32).astype(ml_dtypes.bfloat16)
    assert x.shape == (H, W)
    w_flat = _band_weights(np.asarray(distance_weights, dtype=np.float32))
    xpad = np.concatenate([x[:, -MAXD:], x, x[:, :MAXD]], axis=1)
    in_maps = []
    for c in range(N_CORES):
        rows = np.arange(c * ROWS_PER_CORE - MAXD,
                         c * ROWS_PER_CORE + ROWS_PER_CORE + MAXD) % H
        in_maps.append({"x": np.ascontiguousarray(xpad[rows]), "w": w_flat})
    return in_maps


def kernel(grid_spikes: np.ndarray, distance_weights: np.ndarray) -> np.ndarray:
    if "nc" not in _CACHE:
        _CACHE["nc"] = _build()
    nc = _CACHE["nc"]
    in_maps = make_in_maps(grid_spikes, distance_weights)
    res = run_bass_kernel_spmd(nc, in_maps, list(range(N_CORES)))
    out = np.concatenate([res.results[c]["y"] for c in range(N_CORES)], axis=0)
    return out.astype(np.float32)


# revision 24
# speedup vs baseline: 1.2098x; 1.0304x over previous
"""Trainium2 Bass kernel for nn_LocalConnectivity (diamond-ring circular stencil).

out[i,j] = sum_{d=1..5} w_d * sum_{|di|+|dj|=d} x[(i+di)%H, (j+dj)%W]

Strategy: row-shard across 8 NeuronCores (512 rows each + 5-row circular
halo, columns pre-padded with 5-col circular halo on host). Per core the
60-tap stencil runs on the TensorEngine as banded matmuls. The kernel is
symmetric in dj, so the DVE/Pool engines pre-fold the column shifts
(S_j = x(c-j) + x(c+j)) and only 6 matmul streams per 512-col chunk are
needed (dj=0 plus folded j=1..5) instead of 11. All matmul operands are
bf16 (1 cycle/row at the full 2.4 GHz PE clock vs fp32r's 1.2 GHz).
Output is written bf16 and upcast on host. Stores are issued per 512-col
chunk (strided DRAM destination) so the DGE round-robins packets across
all 16 SDMA engines instead of chaining the whole window on one.
"""
import numpy as np
import ml_dtypes
from contextlib import ExitStack

import concourse.bass as bass
import concourse.tile as tile
from concourse import bacc, mybir
from concourse.bass_utils import run_bass_kernel_spmd

N_CORES = 8
H = W = 4096
MAXD = 5
ROWS_PER_CORE = H // N_CORES          # 512
IN_ROWS = ROWS_PER_CORE + 2 * MAXD    # 522
IN_COLS = W + 2 * MAXD                # 4106
NCOL = 512                            # matmul free dim (one PSUM bank, fp32 max)
NCHUNK = W // NCOL                    # 8
M_OUT = 118                           # output rows per row-window (K=128 - 2*MAXD)
NJ = MAXD + 1                         # dj=0 plus folded |dj|=1..5
# row windows: (input_row_start, out_row_start, K, M)
WINDOWS = []
_o = 0
while _o < ROWS_PER_CORE:
    m = min(M_OUT, ROWS_PER_CORE - _o)
    WINDOWS.append((_o, _o, m + 2 * MAXD, m))
    _o += m

_CACHE = {}


def _band_weights(distance_weights: np.ndarray) -> np.ndarray:
    """w_flat [128, 6*118] bf16: w_flat[k, j*118 + m] = K2d[k-m-5, j].

    Column block j holds the vertical taps for |dj|=j (the dj fold uses
    K2d[di, dj] == K2d[di, -dj], so one band serves both signs)."""
    wd = np.asarray(distance_weights, dtype=np.float32)
    w = np.zeros((NJ, 128, M_OUT), dtype=np.float32)
    for dj in range(0, MAXD + 1):
        for di in range(-MAXD, MAXD + 1):
            d = abs(di) + dj
            if not (1 <= d <= MAXD):
                continue
            m = np.arange(M_OUT)
            k = m + MAXD + di
            ok = (k >= 0) & (k < 128)
            w[dj, k[ok], m[ok]] = wd[d - 1]
    out = np.ascontiguousarray(w.transpose(1, 0, 2).reshape(128, NJ * M_OUT))
    return out.astype(ml_dtypes.bfloat16)


def _build():
    dtb = mybir.dt.bfloat16
    dtf = mybir.dt.float32
    nc = bacc.Bacc("TRN2", target_bir_lowering=False, debug=False,
                   num_devices=N_CORES)
    x = nc.dram_tensor("x", [IN_ROWS, IN_COLS], dtb, kind="ExternalInput").ap()
    wts = nc.dram_tensor("w", [128, NJ * M_OUT], dtb, kind="ExternalInput").ap()
    y = nc.dram_tensor("y", [ROWS_PER_CORE, W], dtb, kind="ExternalOutput").ap()

    with tile.TileContext(nc) as tc, ExitStack() as ctx:
        spool = ctx.enter_context(tc.tile_pool(name="strip", bufs=3))
        fpool = ctx.enter_context(tc.tile_pool(name="folds", bufs=2))
        wpool = ctx.enter_context(tc.tile_pool(name="wts", bufs=1))
        ocpool = ctx.enter_context(tc.tile_pool(name="outc", bufs=16))
        ppool = ctx.enter_context(tc.tile_pool(name="ps", bufs=8, space="PSUM"))

        # Strip loads ride the scalar HWDGE queue: one contiguous full-row
        # transfer per window; row packets round-robin over 16 SDMA engines.
        strips = {}
        all_folds = {}

        def load_strip(wi, engine=None):
            in0, _, kdim, _ = WINDOWS[wi]
            st = spool.tile([128, IN_COLS], dtb, tag="strip")
            (engine or nc.scalar).dma_start(st[:kdim, :], x[in0:in0 + kdim, :])
            strips[wi] = st

        def fold(wi, maxj=MAXD):
            # Column folds S_j = x(c-j) + x(c+j), all on DVE (a concurrent
            # Pool-engine tensor op would contend on the SBUF bus and slow
            # both engines ~4x). Issued one window ahead so the PE rarely
            # waits. Windows 0/1 skip the S_5 fold (the PE runs +-5 streams
            # instead): DVE is saturated from strip0-arrival through window
            # 1, and shedding 2.28us of fold there beats the 1.73us extra
            # stream on the PE.
            st = strips[wi]
            in0, _, kdim, _ = WINDOWS[wi]
            fs = [st]
            for j in range(1, maxj + 1):
                sj = fpool.tile([128, W], dtb, tag=f"s{j}")
                nc.vector.tensor_add(sj[:kdim, :],
                                     st[:kdim, MAXD - j:MAXD - j + W],
                                     st[:kdim, MAXD + j:MAXD + j + W])
                fs.append(sj)
            all_folds[wi] = fs

        load_strip(0)
        wt = wpool.tile([128, NJ * M_OUT], dtb)
        nc.sync.dma_start(wt[:], wts[:])
        load_strip(1, engine=nc.gpsimd)
        fold(0, maxj=MAXD - 1)

        NW = len(WINDOWS)
        for wi, (in0, out0, kdim, m) in enumerate(WINDOWS):
            st = strips.pop(wi)
            if wi + 2 < NW:
                load_strip(wi + 2)
            if wi + 1 < NW:
                fold(wi + 1, maxj=MAXD - 1 if wi == 0 else MAXD)
            folds = all_folds.pop(wi)
            last = wi == NW - 1
            # (j, source-slice) streams per chunk. Windows 0/1: j=5 runs as
            # a +-5 stream pair off the raw strip (the folded band serves
            # both signs since K2d is dj-symmetric).
            def streams(c0):
                ss = [(0, st[:kdim, MAXD + c0:MAXD + c0 + NCOL])]
                for j in range(1, len(folds)):
                    ss.append((j, folds[j][:kdim, c0:c0 + NCOL]))
                if len(folds) == MAXD:
                    ss.append((MAXD, st[:kdim, c0:c0 + NCOL]))
                    ss.append((MAXD, st[:kdim, 2 * MAXD + c0:
                                        2 * MAXD + c0 + NCOL]))
                return ss
            pss = [ppool.tile([m, NCOL], dtf, tag="ps", name=f"ps{cc}")
                   for cc in range(NCHUNK)]
            if wi <= 1:
                # dj-outer: each j-pass (8 chunks, ~1.8us) consumes S_j
                # right as DVE finishes folding it -- the PE trails the fold
                # pipeline by one j instead of waiting for the whole fold
                # set. All 8 PSUM banks accumulate at once.
                chunk_streams = [streams(cc * NCOL) for cc in range(NCHUNK)]
                ns = len(chunk_streams[0])
                for si in range(ns):
                    for cc in range(NCHUNK):
                        j, src = chunk_streams[cc][si]
                        nc.tensor.matmul(
                            pss[cc][:], wt[:kdim, j * M_OUT:j * M_OUT + m],
                            src, start=(si == 0), stop=(si == ns - 1),
                            skip_group_check=True,
                        )
            else:
                for cc in range(NCHUNK):
                    ss = streams(cc * NCOL)
                    for si, (j, src) in enumerate(ss):
                        nc.tensor.matmul(
                            pss[cc][:], wt[:kdim, j * M_OUT:j * M_OUT + m],
                            src, start=(si == 0), stop=(si == len(ss) - 1),
                        )
            for cc2 in range(0, NCHUNK, 2):
                # Copy two PSUM banks into one [m, 1024] tile and store both
                # at once: 2KB-row store packets double SDMA efficiency and
                # halve the SWDGE descriptor-generation load on Pool. In the
                # last window DVE is done folding, so it takes half the
                # copies to cut the drain tail.
                ob = ocpool.tile([m, 2 * NCOL], dtb, tag="out")
                for h in range(2):
                    if last and h == 1:
                        nc.vector.tensor_copy(
                            ob[:, h * NCOL:(h + 1) * NCOL], pss[cc2 + h][:])
                    else:
                        nc.scalar.copy(
                            ob[:, h * NCOL:(h + 1) * NCOL], pss[cc2 + h][:])
                nc.gpsimd.dma_start(
                    y[out0:out0 + m, cc2 * NCOL:(cc2 + 2) * NCOL], ob[:])
    nc.compile()
    return nc


def make_in_maps(grid_spikes: np.ndarray, distance_weights: np.ndarray):
    x = np.asarray(grid_spikes, dtype=np.float32).astype(ml_dtypes.bfloat16)
    assert x.shape == (H, W)
    w_flat = _band_weights(np.asarray(distance_weights, dtype=np.float32))
    xpad = np.concatenate([x[:, -MAXD:], x, x[:, :MAXD]], axis=1)
    in_maps = []
    for c in range(N_CORES):
        rows = np.arange(c * ROWS_PER_CORE - MAXD,
                         c * ROWS_PER_CORE + ROWS_PER_CORE + MAXD) % H
        in_maps.append({"x": np.ascontiguousarray(xpad[rows]), "w": w_flat})
    return in_maps


def kernel(grid_spikes: np.ndarray, distance_weights: np.ndarray) -> np.ndarray:
    if "nc" not in _CACHE:
        _CACHE["nc"] = _build()
    nc = _CACHE["nc"]
    in_maps = make_in_maps(grid_spikes, distance_weights)
    res = run_bass_kernel_spmd(nc, in_maps, list(range(N_CORES)))
    out = np.concatenate([res.results[c]["y"] for c in range(N_CORES)], axis=0)
    return out.astype(np.float32)
